# revision 31
# baseline (speedup 1.0000x reference)
"""GCNConv (gnn_message_passing) Trainium2 kernel — 8 NeuronCores, Bass/Tile.

Computes  out = segment_sum_dst(edge_vals * x[edge_src]) @ W + bias
for N=100000 nodes, E=3.2M edges, F=256, as fp32 in / fp32 out.

Strategy (all hardcoded for the 100000x256 / 3.2M-edge shape family):
  - Destination nodes are sharded over the 8 cores (12500 rows each); every
    core receives the full x in its HBM (staged by the runner, not kernel
    time) and produces its 12500 output rows; the host reassembles.
  - Per core, edges are grouped by (dst block of 128, src chunk of 25000)
    — the chunk split keeps gather indices within int16 range — sorted, and
    padded to 128-edge tiles.  Superblocks of SB dst blocks share chunk runs;
    chunk order snakes (even sb: 0..3, odd sb: 3..0) so adjacent superblocks
    meet on the same chunk and gather chains pack across the boundary.
    The tile schedule is the max across cores so one SPMD program serves all
    8 cores; per-core blocks are rank-matched by edge count.
  - Per chain (up to 8 tiles, 1024 edges, one src chunk):
      msgs[e, f]  = dma_gather(x_chunk, src[e])          (SWDGE, 4 queues)
  - A-tiles (A[e, d] = vals[e] * (iota[d] == dstloc[e])) are built 32 tiles
    at a time with two DVE tensor_tensor ops over broadcast access patterns
    (is_equal then mult) — ~90ns/tile vs 660ns/tile for per-tile
    tensor_scalar.
  - Per tile: psum[d, f] += A.T @ msgs on PE (bf16 in, fp32 acc); when a
    block's last tile lands, its epilogue runs immediately:
    support -> PE transpose -> supportT.T @ W (fp32r) + bias -> 128 rows out.
  - x and A are bf16 (gather traffic and DVE rate dominate; measured
    end-to-end error vs the fp32 reference is ~2e-3 of the output absmax).
"""
import os
import sys

sys.path.insert(0, '/opt/trn_rl_repo')
from contextlib import ExitStack

import ml_dtypes
import numpy as np

import concourse.bacc as bacc
import concourse.tile as tile
import concourse.mybir as mybir

F32 = mybir.dt.float32
F32R = mybir.dt.float32r
BF16 = mybir.dt.bfloat16
I16 = mybir.dt.int16
P = 128
GMAX = 8          # dma_gather ucode limit: 1024 idxs (64 per Q7 lane)
AluOp = mybir.AluOpType

# ---- fixed problem/config constants -------------------------------------
N_NODES = 100000
F_IN = 256
F_OUT = 256
N_CORES = 8
CHUNK = 25000             # int16 gather-index range
NPC = N_NODES // N_CORES  # 12500 dst rows per core
NBLK = (NPC + P - 1) // P  # 98 blocks per core
CH = (N_NODES + CHUNK - 1) // CHUNK  # 4 src chunks
SB = 4                    # superblock: blocks whose chunk runs merge
ABATCH = 16               # tiles per batched DVE A-build
N_QUEUES = 4
MSG_BUFS = 8
AB_BUFS = 3
T1_BUFS = 2
IDX_SLICES = 8


def _make_schedule(counts_slot):
    """counts_slot: [NC, NBLK, CH] -> (runs, chains, tile_pieces, NT, NPIECE).

    runs: (ch, slots, R, first_tile) per (superblock, chunk) in snake order.
    tile_pieces: per global tile, list of (piece_col, slot).
    chains: (chunk, first_tile, ntiles) with 8/7 alternation.
    """
    runs = []
    tile_ch = []
    tile_pieces = []
    for isb, sb0 in enumerate(range(0, NBLK, SB)):
        slots = list(range(sb0, min(sb0 + SB, NBLK)))
        chs = range(CH) if isb % 2 == 0 else range(CH - 1, -1, -1)
        for ch in chs:
            rc = counts_slot[:, slots, ch]          # [NC, nslots]
            cum = np.zeros((rc.shape[0], len(slots) + 1), dtype=np.int64)
            np.cumsum(rc, axis=1, out=cum[:, 1:])
            R = int(np.ceil(cum[:, -1].max() / P))
            first = len(tile_ch)
            for _ in range(R):
                tile_ch.append(ch)
                tile_pieces.append([])
            for k, slot in enumerate(slots):
                t_lo = int(cum[:, k].min()) // P
                t_hi = min(R, -(-int(cum[:, k + 1].max()) // P))
                for t in range(t_lo, t_hi):
                    tile_pieces[first + t].append(slot)
            runs.append((ch, tuple(slots), R, first))
    NT = len(tile_ch)
    col = 0
    for gt in range(NT):
        tile_pieces[gt] = [(col + i, s) for i, s in
                           enumerate(tile_pieces[gt])]
        col += len(tile_pieces[gt])
    NPIECE = col
    # 8/7-tile chains: same-queue in-flight pairs stay at 65+57=122
    # descriptors per engine, under the 128-slot SWDGE ring.
    chains = []
    i = 0
    while i < NT:
        ch = tile_ch[i]
        cap = GMAX if len(chains) % 2 == 0 else GMAX - 1
        j = i
        while j < NT and j - i < cap and tile_ch[j] == ch:
            j += 1
        chains.append((ch, i, j - i))
        i = j
    return runs, chains, tile_pieces, NT, NPIECE


def _preprocess(edge_src, edge_dst, edge_vals):
    edge_src = np.asarray(edge_src).astype(np.int64)
    edge_dst = np.asarray(edge_dst).astype(np.int64)
    edge_vals = np.asarray(edge_vals, dtype=np.float32)
    core = edge_dst // NPC
    dloc = edge_dst % NPC
    b = dloc // P
    j = dloc % P
    ch = edge_src // CHUNK
    s = edge_src % CHUNK

    gid = (core * NBLK + b) * CH + ch
    ngroups = N_CORES * NBLK * CH
    counts = np.bincount(gid, minlength=ngroups).reshape(N_CORES, NBLK, CH)
    # rank-match blocks to slots by per-core total count
    totals = counts.sum(axis=2)
    perms = np.argsort(-totals, axis=1, kind='stable')
    counts_slot = np.take_along_axis(counts, perms[:, :, None], axis=1)
    runs, chains, tile_pieces, NT, NPIECE = _make_schedule(counts_slot)
    NPP = ((NPIECE + ABATCH - 1) // ABATCH) * ABATCH

    sort_idx = np.argsort(gid, kind='stable')
    s_sorted = s[sort_idx].astype(np.int16)
    j_sorted = j[sort_idx].astype(np.float32)
    v_sorted = edge_vals[sort_idx]
    group_starts = np.zeros(ngroups + 1, dtype=np.int64)
    np.cumsum(counts.reshape(-1), out=group_starts[1:])

    per_core = []
    for c in range(N_CORES):
        IDX = np.zeros((P, NT * 8), dtype=np.int16)
        DL = np.full((P, NPP), -1.0, dtype=ml_dtypes.bfloat16)
        VL = np.zeros((P, NPP), dtype=ml_dtypes.bfloat16)
        for (cc, slots, R, first) in runs:
            L = R * P
            run_s = np.zeros(L, dtype=np.int16)
            run_j = np.full(L, -1.0, dtype=np.float32)
            run_v = np.zeros(L, dtype=np.float32)
            run_k = np.full(L, -1, dtype=np.int16)
            n = 0
            for k, slot in enumerate(slots):
                g = (c * NBLK + int(perms[c, slot])) * CH + cc
                e0, e1 = group_starts[g], group_starts[g + 1]
                m = int(e1 - e0)
                run_s[n:n + m] = s_sorted[e0:e1]
                run_j[n:n + m] = j_sorted[e0:e1]
                run_v[n:n + m] = v_sorted[e0:e1]
                run_k[n:n + m] = k
                n += m
            for t in range(R):
                gt = first + t
                sl = run_s[t * P:(t + 1) * P]
                IDX[:, gt * 8:(gt + 1) * 8] = \
                    np.tile(sl.reshape(8, 16).T, (8, 1))
                jseg = run_j[t * P:(t + 1) * P]
                vseg = run_v[t * P:(t + 1) * P]
                kseg = run_k[t * P:(t + 1) * P]
                for (col, slot) in tile_pieces[gt]:
                    k = slots.index(slot)
                    m = kseg == k
                    DL[:, col] = np.where(m, jseg, -1.0).astype(
                        ml_dtypes.bfloat16)
                    VL[:, col] = np.where(m, vseg, 0.0).astype(
                        ml_dtypes.bfloat16)
        per_core.append((IDX, DL, VL, VL.astype(np.float32)))
    return runs, chains, tile_pieces, NT, NPP, per_core, perms


def _build_program(chains, tile_pieces, NT, NPP):
    nc = bacc.Bacc("TRN2", debug=False, target_bir_lowering=False,
                   num_swdge_queues=N_QUEUES)
    x_d = nc.dram_tensor("x", [N_NODES, F_IN], BF16, kind="ExternalInput").ap()
    w_d = nc.dram_tensor("w", [F_IN, F_OUT], F32, kind="ExternalInput").ap()
    iota_d = nc.dram_tensor("iota", [P, P], BF16, kind="ExternalInput").ap()
    ident_d = nc.dram_tensor("ident", [P, P], F32, kind="ExternalInput").ap()
    biasb_d = nc.dram_tensor("biasb", [P, F_OUT], F32, kind="ExternalInput").ap()
    idx_d = nc.dram_tensor("idx", [P, NT * 8], I16, kind="ExternalInput").ap()
    dl_d = nc.dram_tensor("dstloc", [P, NPP], BF16, kind="ExternalInput").ap()
    vl_d = nc.dram_tensor("vals", [P, NPP], BF16, kind="ExternalInput").ap()
    vl32_d = nc.dram_tensor("vals32", [P, NPP], F32, kind="ExternalInput").ap()
    out_d = nc.dram_tensor("out", [NBLK * P, F_OUT], F32,
                           kind="ExternalOutput").ap()
    KT = F_IN // P
    ntb = np.zeros(NBLK, dtype=np.int64)
    for pl in tile_pieces:
        for (_, slot) in pl:
            ntb[slot] += 1

    with tile.TileContext(nc) as tc, ExitStack() as ctx:
        const = ctx.enter_context(tc.tile_pool(name="const", bufs=1))
        IDX = const.tile([P, NT * 8], I16)
        step = ((NT * 8 // IDX_SLICES) // 8 + 1) * 8
        ib = [0, 256] + list(range(step, NT * 8, step)) + [NT * 8]
        for k, hi in zip(ib, ib[1:]):
            if hi > k:
                nc.sync.dma_start(IDX[:, k:hi], idx_d[:, k:hi])
        DL = const.tile([P, NPP], BF16)
        VL = const.tile([P, NPP], BF16)
        VL32 = const.tile([P, NPP], F32)
        qstep = max(ABATCH, (NPP // 4 // ABATCH) * ABATCH)
        qb = list(range(0, NPP, qstep)) + [NPP]
        for lo, hi in zip(qb, qb[1:]):
            if hi > lo:
                nc.sync.dma_start(DL[:, lo:hi], dl_d[:, lo:hi])
                nc.sync.dma_start(VL[:, lo:hi], vl_d[:, lo:hi])
                nc.sync.dma_start(VL32[:, lo:hi], vl32_d[:, lo:hi])
        IOTA = const.tile([P, P], BF16)
        nc.sync.dma_start(IOTA[:], iota_d[:])
        IDENT = const.tile([P, P], F32R)
        nc.sync.dma_start(IDENT[:], ident_d[:].bitcast(F32R))
        BIASB = const.tile([P, F_OUT], F32)
        nc.sync.dma_start(BIASB[:], biasb_d[:])
        Wt = []
        for k in range(KT):
            wk = const.tile([P, F_OUT], F32R, tag=f"w{k}", name=f"w{k}")
            nc.sync.dma_start(wk[:], w_d[k * P:(k + 1) * P, :].bitcast(F32R))
            Wt.append(wk)

        gp = ctx.enter_context(tc.tile_pool(name="msgs", bufs=MSG_BUFS))
        abpool = ctx.enter_context(tc.tile_pool(name="abp", bufs=AB_BUFS))
        t1pool = ctx.enter_context(tc.tile_pool(name="t1p", bufs=T1_BUFS))
        ep = ctx.enter_context(tc.tile_pool(name="epil", bufs=2))
        ps_s = ctx.enter_context(
            tc.tile_pool(name="ps_s", bufs=SB + 1, space="PSUM"))
        ps_t = ctx.enter_context(tc.tile_pool(name="ps_t", bufs=1, space="PSUM"))
        ps_o = ctx.enter_context(tc.tile_pool(name="ps_o", bufs=2, space="PSUM"))

        psum_of = {}
        mm_count = {}
        gath_q = 0
        ab_tiles = {}

        HALF = ABATCH // 2
        IDENT_F = mybir.ActivationFunctionType.Identity

        def ensure_ab(gidx):
            if gidx in ab_tiles:
                return ab_tiles[gidx]
            lo = gidx * ABATCH
            t1 = t1pool.tile([P, ABATCH, P], BF16, tag="t1", name="t1")
            ab = abpool.tile([P, ABATCH, P], BF16, tag="ab", name="ab")
            nc.vector.tensor_tensor(
                t1[:], IOTA[:].unsqueeze(1).broadcast_to([P, ABATCH, P]),
                DL[:, lo:lo + ABATCH].unsqueeze(2).broadcast_to([P, ABATCH, P]),
                op=AluOp.is_equal)
            # VL-scale: first half batched on DVE, second half as per-tile
            # Identity activations with per-partition scale on the idle ACT.
            nc.vector.tensor_tensor(
                ab[:, :HALF, :], t1[:, :HALF, :],
                VL[:, lo:lo + HALF].unsqueeze(2).broadcast_to([P, HALF, P]),
                op=AluOp.mult)
            for i in range(HALF, ABATCH):
                nc.scalar.activation(ab[:, i, :], t1[:, i, :], IDENT_F,
                                     scale=VL32[:, lo + i:lo + i + 1])
            ab_tiles[gidx] = ab
            return ab

        def epilogue(b):
            psum_s = psum_of.pop(b)
            s_sb = ep.tile([P, F_IN], F32R, tag="s_sb", name="s_sb")
            nc.scalar.copy(s_sb[:], psum_s[:])
            # init the output psum with the bias (ACT write); the W-matmuls
            # accumulate on top (start=False), keeping the add off the DVE.
            outp = ps_o.tile([P, F_OUT], F32, tag="outp", name="outp")
            nc.scalar.copy(outp[:], BIASB[:])
            for h in range(KT):
                pt = ps_t.tile([P, P], F32R, tag="pt", name="pt")
                nc.tensor.transpose(pt[:], s_sb[:, h * P:(h + 1) * P], IDENT[:])
                sth = ep.tile([P, P], F32R, tag="sth", name="sth")
                nc.scalar.copy(sth[:], pt[:])
                nc.tensor.matmul(outp[:], sth[:], Wt[h][:],
                                 start=False, stop=(h == KT - 1))
            ob = ep.tile([P, F_OUT], F32, tag="ob", name="ob")
            nc.scalar.copy(ob[:], outp[:])
            nc.sync.dma_start(out_d[b * P:(b + 1) * P, :], ob[:])

        for (ch, t0, n) in chains:
            hi = min((ch + 1) * CHUNK, N_NODES)
            g = gp.tile([P, n, F_IN], BF16, tag="msgs", name="msgs")
            nc.gpsimd.dma_gather(
                g[:], x_d[ch * CHUNK:hi, :], IDX[:, t0 * 8:(t0 + n) * 8],
                n * P, n * P, F_IN, queue_num=gath_q % N_QUEUES,
            )
            gath_q += 1
            for t in range(n):
                gt = t0 + t
                for (col, b) in tile_pieces[gt]:
                    ab = ensure_ab(col // ABATCH)
                    if b not in psum_of:
                        psum_of[b] = ps_s.tile([P, F_IN], F32, tag="psum_s",
                                               name=f"psum_s{b}")
                        mm_count[b] = 0
                    nc.tensor.matmul(
                        psum_of[b][:], ab[:, col % ABATCH, :], g[:, t, :],
                        start=(mm_count[b] == 0),
                        stop=(mm_count[b] == int(ntb[b]) - 1))
                    mm_count[b] += 1
                    if mm_count[b] == int(ntb[b]):
                        epilogue(b)
        assert not psum_of, list(psum_of)

    nc.compile()
    return nc


def _install_profile_shim():
    """antenv.axon_hooks is absent in this image; recreate it so
    run_bass_kernel_spmd(trace=True) can NTFF-profile under axon."""
    import types
    if "antenv.axon_hooks" in sys.modules:
        return
    import antenv
    mod = types.ModuleType("antenv.axon_hooks")
    mod._hook = None

    def set_axon_ntff_profile_hook(h):
        mod._hook = h

    def get_axon_ntff_profile_hook():
        if mod._hook is None:
            try:
                from trn_agent_boot.trn_boot import _ntff_profile_via_ctypes
                mod._hook = _ntff_profile_via_ctypes('/opt/axon/libaxon_pjrt.so')
            except Exception:
                return None
        return mod._hook

    mod.set_axon_ntff_profile_hook = set_axon_ntff_profile_hook
    mod.get_axon_ntff_profile_hook = get_axon_ntff_profile_hook
    sys.modules["antenv.axon_hooks"] = mod
    antenv.axon_hooks = mod


_PROGRAM_CACHE = {}


def kernel(x, edge_src, edge_dst, edge_vals, W, bias):
    x = np.asarray(x, dtype=np.float32)
    W = np.asarray(W, dtype=np.float32)
    bias = np.asarray(bias, dtype=np.float32)
    assert x.shape == (N_NODES, F_IN), x.shape

    runs, chains, tile_pieces, NT, NPP, per_core, perms = _preprocess(
        edge_src, edge_dst, edge_vals)

    key = (NT, NPP, tuple(chains),
           tuple(tuple(pl) for pl in tile_pieces))
    if key not in _PROGRAM_CACHE:
        _PROGRAM_CACHE.clear()
        _PROGRAM_CACHE[key] = _build_program(chains, tile_pieces, NT, NPP)
    nc = _PROGRAM_CACHE[key]

    x_bf = x.astype(ml_dtypes.bfloat16)
    iota = np.broadcast_to(np.arange(P).astype(ml_dtypes.bfloat16),
                           (P, P)).copy()
    ident = np.eye(P, dtype=np.float32)
    biasb = np.broadcast_to(bias, (P, F_OUT)).copy()
    maps = []
    for c in range(N_CORES):
        IDX, DL, VL, VL32 = per_core[c]
        maps.append({"x": x_bf, "w": W, "iota": iota, "ident": ident,
                     "biasb": biasb, "idx": IDX, "dstloc": DL, "vals": VL,
                     "vals32": VL32})

    trace = os.environ.get("GCN_KERNEL_TRACE", "0") == "1"
    if trace:
        _install_profile_shim()
    from concourse.bass_utils import run_bass_kernel_spmd
    res = run_bass_kernel_spmd(nc, maps, list(range(N_CORES)), trace=trace)
    if trace and res.exec_time_ns is not None:
        print(f"HW exec time: {res.exec_time_ns} ns")

    out = np.empty((N_NODES, F_OUT), dtype=np.float32)
    for c in range(N_CORES):
        r = res.results[c]["out"]
        for s in range(NBLK):
            blk = int(perms[c, s])
            rows = min(P, NPC - blk * P)
            out[c * NPC + blk * P: c * NPC + blk * P + rows, :] = \
                r[s * P: s * P + rows, :]
    return out


# revision 33
# speedup vs baseline: 1.1407x; 1.1407x over previous
"""GCNConv (gnn_message_passing) Trainium2 kernel — 8 NeuronCores, Bass/Tile.

Computes  out = segment_sum_dst(edge_vals * x[edge_src]) @ W + bias
for N=100000 nodes, E=3.2M edges, F=256, as fp32 in / fp32 out.

Strategy (all hardcoded for the 100000x256 / 3.2M-edge shape family):
  - Destination nodes are sharded over the 8 cores (12500 rows each); every
    core receives the full x in its HBM (staged by the runner, not kernel
    time) and produces its 12500 output rows; the host reassembles.
  - Per core, edges are grouped by (dst block of 128, src chunk of 25000)
    — the chunk split keeps gather indices within int16 range — sorted, and
    padded to 128-edge tiles.  Superblocks of SB dst blocks share chunk runs;
    chunk order snakes (even sb: 0..3, odd sb: 3..0) so adjacent superblocks
    meet on the same chunk and gather chains pack across the boundary.
    The tile schedule is the max across cores so one SPMD program serves all
    8 cores; per-core blocks are rank-matched by edge count.
  - Per chain (up to 8 tiles, 1024 edges, one src chunk):
      msgs[e, f]  = dma_gather(x_chunk, src[e])          (SWDGE, 4 queues)
  - A-tiles (A[e, d] = vals[e] * (iota[d] == dstloc[e])) are built 32 tiles
    at a time with two DVE tensor_tensor ops over broadcast access patterns
    (is_equal then mult) — ~90ns/tile vs 660ns/tile for per-tile
    tensor_scalar.
  - Per tile: psum[d, f] += A.T @ msgs on PE (bf16 in, fp32 acc); when a
    block's last tile lands, its epilogue runs immediately:
    support -> PE transpose -> supportT.T @ W (fp32r) + bias -> 128 rows out.
  - x and A are bf16 (gather traffic and DVE rate dominate; measured
    end-to-end error vs the fp32 reference is ~2e-3 of the output absmax).
"""
import os
import sys

sys.path.insert(0, '/opt/trn_rl_repo')
from contextlib import ExitStack

import ml_dtypes
import numpy as np

import concourse.bacc as bacc
import concourse.tile as tile
import concourse.mybir as mybir

F32 = mybir.dt.float32
F32R = mybir.dt.float32r
BF16 = mybir.dt.bfloat16
I16 = mybir.dt.int16
P = 128
GMAX = 8          # dma_gather ucode limit: 1024 idxs (64 per Q7 lane)
AluOp = mybir.AluOpType

# ---- fixed problem/config constants -------------------------------------
N_NODES = 100000
F_IN = 256
F_OUT = 256
N_CORES = 8
CHUNK = 25000             # int16 gather-index range
NPC = N_NODES // N_CORES  # 12500 dst rows per core
NBLK = (NPC + P - 1) // P  # 98 blocks per core
CH = (N_NODES + CHUNK - 1) // CHUNK  # 4 src chunks
SB = 4                    # superblock: blocks whose chunk runs merge
ABATCH = 16               # tiles per batched DVE A-build
N_QUEUES = 4
MSG_BUFS = 12
AB_BUFS = 3
T1_BUFS = 2
IDX_SLICES = 8


def _make_schedule(counts_slot):
    """counts_slot: [NC, NBLK, CH] -> (runs, chains, tile_pieces, NT, NPIECE).

    runs: (ch, slots, R, first_tile) per (superblock, chunk) in snake order.
    tile_pieces: per global tile, list of (piece_col, slot).
    chains: (chunk, first_tile, ntiles) with 8/7 alternation.
    """
    runs = []
    tile_ch = []
    tile_pieces = []
    for isb, sb0 in enumerate(range(0, NBLK, SB)):
        slots = list(range(sb0, min(sb0 + SB, NBLK)))
        chs = range(CH) if isb % 2 == 0 else range(CH - 1, -1, -1)
        for ch in chs:
            rc = counts_slot[:, slots, ch]          # [NC, nslots]
            cum = np.zeros((rc.shape[0], len(slots) + 1), dtype=np.int64)
            np.cumsum(rc, axis=1, out=cum[:, 1:])
            R = int(np.ceil(cum[:, -1].max() / P))
            first = len(tile_ch)
            for _ in range(R):
                tile_ch.append(ch)
                tile_pieces.append([])
            for k, slot in enumerate(slots):
                t_lo = int(cum[:, k].min()) // P
                t_hi = min(R, -(-int(cum[:, k + 1].max()) // P))
                for t in range(t_lo, t_hi):
                    tile_pieces[first + t].append(slot)
            runs.append((ch, tuple(slots), R, first))
    NT = len(tile_ch)
    col = 0
    for gt in range(NT):
        tile_pieces[gt] = [(col + i, s) for i, s in
                           enumerate(tile_pieces[gt])]
        col += len(tile_pieces[gt])
    NPIECE = col
    # 8/7-tile chains: same-queue in-flight pairs stay at 65+57=122
    # descriptors per engine, under the 128-slot SWDGE ring.
    chains = []
    i = 0
    while i < NT:
        ch = tile_ch[i]
        cap = GMAX if len(chains) % 2 == 0 else GMAX - 1
        j = i
        while j < NT and j - i < cap and tile_ch[j] == ch:
            j += 1
        chains.append((ch, i, j - i))
        i = j
    return runs, chains, tile_pieces, NT, NPIECE


def _preprocess(edge_src, edge_dst, edge_vals):
    edge_src = np.asarray(edge_src).astype(np.int64)
    edge_dst = np.asarray(edge_dst).astype(np.int64)
    edge_vals = np.asarray(edge_vals, dtype=np.float32)
    core = edge_dst // NPC
    dloc = edge_dst % NPC
    b = dloc // P
    j = dloc % P
    ch = edge_src // CHUNK
    s = edge_src % CHUNK

    gid = (core * NBLK + b) * CH + ch
    ngroups = N_CORES * NBLK * CH
    counts = np.bincount(gid, minlength=ngroups).reshape(N_CORES, NBLK, CH)
    # rank-match blocks to slots by per-core total count
    totals = counts.sum(axis=2)
    perms = np.argsort(-totals, axis=1, kind='stable')
    counts_slot = np.take_along_axis(counts, perms[:, :, None], axis=1)
    runs, chains, tile_pieces, NT, NPIECE = _make_schedule(counts_slot)
    NPP = ((NPIECE + ABATCH - 1) // ABATCH) * ABATCH

    sort_idx = np.argsort(gid, kind='stable')
    s_sorted = s[sort_idx].astype(np.int16)
    j_sorted = j[sort_idx].astype(np.float32)
    v_sorted = edge_vals[sort_idx]
    group_starts = np.zeros(ngroups + 1, dtype=np.int64)
    np.cumsum(counts.reshape(-1), out=group_starts[1:])

    per_core = []
    for c in range(N_CORES):
        IDX = np.zeros((P, NT * 8), dtype=np.int16)
        DL = np.full((P, NPP), -1.0, dtype=ml_dtypes.bfloat16)
        VL = np.zeros((P, NPP), dtype=ml_dtypes.bfloat16)
        for (cc, slots, R, first) in runs:
            L = R * P
            run_s = np.zeros(L, dtype=np.int16)
            run_j = np.full(L, -1.0, dtype=np.float32)
            run_v = np.zeros(L, dtype=np.float32)
            run_k = np.full(L, -1, dtype=np.int16)
            n = 0
            for k, slot in enumerate(slots):
                g = (c * NBLK + int(perms[c, slot])) * CH + cc
                e0, e1 = group_starts[g], group_starts[g + 1]
                m = int(e1 - e0)
                run_s[n:n + m] = s_sorted[e0:e1]
                run_j[n:n + m] = j_sorted[e0:e1]
                run_v[n:n + m] = v_sorted[e0:e1]
                run_k[n:n + m] = k
                n += m
            for t in range(R):
                gt = first + t
                sl = run_s[t * P:(t + 1) * P]
                IDX[:, gt * 8:(gt + 1) * 8] = \
                    np.tile(sl.reshape(8, 16).T, (8, 1))
                jseg = run_j[t * P:(t + 1) * P]
                vseg = run_v[t * P:(t + 1) * P]
                kseg = run_k[t * P:(t + 1) * P]
                for (col, slot) in tile_pieces[gt]:
                    k = slots.index(slot)
                    m = kseg == k
                    DL[:, col] = np.where(m, jseg, -1.0).astype(
                        ml_dtypes.bfloat16)
                    VL[:, col] = np.where(m, vseg, 0.0).astype(
                        ml_dtypes.bfloat16)
        per_core.append((IDX, DL, VL))
    return runs, chains, tile_pieces, NT, NPP, per_core, perms


def _build_program(chains, tile_pieces, NT, NPP):
    nc = bacc.Bacc("TRN2", debug=False, target_bir_lowering=False,
                   num_swdge_queues=N_QUEUES)
    x_d = nc.dram_tensor("x", [N_NODES, F_IN], BF16, kind="ExternalInput").ap()
    w_d = nc.dram_tensor("w", [F_IN, F_OUT], F32, kind="ExternalInput").ap()
    iota_d = nc.dram_tensor("iota", [P, P], BF16, kind="ExternalInput").ap()
    ident_d = nc.dram_tensor("ident", [P, P], F32, kind="ExternalInput").ap()
    biasb_d = nc.dram_tensor("biasb", [P, F_OUT], F32, kind="ExternalInput").ap()
    idx_d = nc.dram_tensor("idx", [P, NT * 8], I16, kind="ExternalInput").ap()
    dl_d = nc.dram_tensor("dstloc", [P, NPP], BF16, kind="ExternalInput").ap()
    vl_d = nc.dram_tensor("vals", [P, NPP], BF16, kind="ExternalInput").ap()
    out_d = nc.dram_tensor("out", [NBLK * P, F_OUT], F32,
                           kind="ExternalOutput").ap()
    KT = F_IN // P
    ntb = np.zeros(NBLK, dtype=np.int64)
    for pl in tile_pieces:
        for (_, slot) in pl:
            ntb[slot] += 1

    with tile.TileContext(nc) as tc, ExitStack() as ctx:
        const = ctx.enter_context(tc.tile_pool(name="const", bufs=1))
        IDX = const.tile([P, NT * 8], I16)
        step = ((NT * 8 // IDX_SLICES) // 8 + 1) * 8
        ib = [0, 256] + list(range(step, NT * 8, step)) + [NT * 8]
        for k, hi in zip(ib, ib[1:]):
            if hi > k:
                nc.sync.dma_start(IDX[:, k:hi], idx_d[:, k:hi])
        DL = const.tile([P, NPP], BF16)
        VL = const.tile([P, NPP], BF16)
        qstep = max(ABATCH, (NPP // 4 // ABATCH) * ABATCH)
        qb = list(range(0, NPP, qstep)) + [NPP]
        for lo, hi in zip(qb, qb[1:]):
            if hi > lo:
                nc.sync.dma_start(DL[:, lo:hi], dl_d[:, lo:hi])
                nc.sync.dma_start(VL[:, lo:hi], vl_d[:, lo:hi])
        IOTA = const.tile([P, P], BF16)
        nc.sync.dma_start(IOTA[:], iota_d[:])
        IDENT = const.tile([P, P], F32R)
        nc.sync.dma_start(IDENT[:], ident_d[:].bitcast(F32R))
        BIASB = const.tile([P, F_OUT], F32)
        nc.sync.dma_start(BIASB[:], biasb_d[:])
        Wt = []
        for k in range(KT):
            wk = const.tile([P, F_OUT], F32R, tag=f"w{k}", name=f"w{k}")
            nc.sync.dma_start(wk[:], w_d[k * P:(k + 1) * P, :].bitcast(F32R))
            Wt.append(wk)

        gp = ctx.enter_context(tc.tile_pool(name="msgs", bufs=MSG_BUFS))
        abpool = ctx.enter_context(tc.tile_pool(name="abp", bufs=AB_BUFS))
        t1pool = ctx.enter_context(tc.tile_pool(name="t1p", bufs=T1_BUFS))
        ep = ctx.enter_context(tc.tile_pool(name="epil", bufs=2))
        ps_s = ctx.enter_context(
            tc.tile_pool(name="ps_s", bufs=SB + 1, space="PSUM"))
        ps_t = ctx.enter_context(tc.tile_pool(name="ps_t", bufs=1, space="PSUM"))
        ps_o = ctx.enter_context(tc.tile_pool(name="ps_o", bufs=2, space="PSUM"))

        psum_of = {}
        mm_count = {}
        gath_q = 0
        ab_tiles = {}

        def ensure_ab(gidx):
            if gidx in ab_tiles:
                return ab_tiles[gidx]
            lo = gidx * ABATCH
            t1 = t1pool.tile([P, ABATCH, P], BF16, tag="t1", name="t1")
            ab = abpool.tile([P, ABATCH, P], BF16, tag="ab", name="ab")
            nc.vector.tensor_tensor(
                t1[:], IOTA[:].unsqueeze(1).broadcast_to([P, ABATCH, P]),
                DL[:, lo:lo + ABATCH].unsqueeze(2).broadcast_to([P, ABATCH, P]),
                op=AluOp.is_equal)
            nc.vector.tensor_tensor(
                ab[:], t1[:],
                VL[:, lo:lo + ABATCH].unsqueeze(2).broadcast_to([P, ABATCH, P]),
                op=AluOp.mult)
            ab_tiles[gidx] = ab
            return ab

        def epilogue(b):
            psum_s = psum_of.pop(b)
            s_sb = ep.tile([P, F_IN], F32R, tag="s_sb", name="s_sb")
            nc.scalar.copy(s_sb[:], psum_s[:])
            # init the output psum with the bias (ACT write); the W-matmuls
            # accumulate on top (start=False), keeping the add off the DVE.
            outp = ps_o.tile([P, F_OUT], F32, tag="outp", name="outp")
            nc.scalar.copy(outp[:], BIASB[:])
            for h in range(KT):
                pt = ps_t.tile([P, P], F32R, tag="pt", name="pt")
                nc.tensor.transpose(pt[:], s_sb[:, h * P:(h + 1) * P], IDENT[:])
                sth = ep.tile([P, P], F32R, tag="sth", name="sth")
                nc.scalar.copy(sth[:], pt[:])
                nc.tensor.matmul(outp[:], sth[:], Wt[h][:],
                                 start=False, stop=(h == KT - 1))
            ob = ep.tile([P, F_OUT], F32, tag="ob", name="ob")
            nc.scalar.copy(ob[:], outp[:])
            nc.sync.dma_start(out_d[b * P:(b + 1) * P, :], ob[:])

        for (ch, t0, n) in chains:
            hi = min((ch + 1) * CHUNK, N_NODES)
            g = gp.tile([P, n, F_IN], BF16, tag="msgs", name="msgs")
            nc.gpsimd.dma_gather(
                g[:], x_d[ch * CHUNK:hi, :], IDX[:, t0 * 8:(t0 + n) * 8],
                n * P, n * P, F_IN, queue_num=gath_q % N_QUEUES,
            )
            gath_q += 1
            for t in range(n):
                gt = t0 + t
                for (col, b) in tile_pieces[gt]:
                    ab = ensure_ab(col // ABATCH)
                    if b not in psum_of:
                        psum_of[b] = ps_s.tile([P, F_IN], F32, tag="psum_s",
                                               name=f"psum_s{b}")
                        mm_count[b] = 0
                    nc.tensor.matmul(
                        psum_of[b][:], ab[:, col % ABATCH, :], g[:, t, :],
                        start=(mm_count[b] == 0),
                        stop=(mm_count[b] == int(ntb[b]) - 1))
                    mm_count[b] += 1
                    if mm_count[b] == int(ntb[b]):
                        epilogue(b)
        assert not psum_of, list(psum_of)

    nc.compile()
    return nc


def _install_profile_shim():
    """antenv.axon_hooks is absent in this image; recreate it so
    run_bass_kernel_spmd(trace=True) can NTFF-profile under axon."""
    import types
    if "antenv.axon_hooks" in sys.modules:
        return
    import antenv
    mod = types.ModuleType("antenv.axon_hooks")
    mod._hook = None

    def set_axon_ntff_profile_hook(h):
        mod._hook = h

    def get_axon_ntff_profile_hook():
        if mod._hook is None:
            try:
                from trn_agent_boot.trn_boot import _ntff_profile_via_ctypes
                mod._hook = _ntff_profile_via_ctypes('/opt/axon/libaxon_pjrt.so')
            except Exception:
                return None
        return mod._hook

    mod.set_axon_ntff_profile_hook = set_axon_ntff_profile_hook
    mod.get_axon_ntff_profile_hook = get_axon_ntff_profile_hook
    sys.modules["antenv.axon_hooks"] = mod
    antenv.axon_hooks = mod


_PROGRAM_CACHE = {}


def kernel(x, edge_src, edge_dst, edge_vals, W, bias):
    x = np.asarray(x, dtype=np.float32)
    W = np.asarray(W, dtype=np.float32)
    bias = np.asarray(bias, dtype=np.float32)
    assert x.shape == (N_NODES, F_IN), x.shape

    runs, chains, tile_pieces, NT, NPP, per_core, perms = _preprocess(
        edge_src, edge_dst, edge_vals)

    key = (NT, NPP, tuple(chains),
           tuple(tuple(pl) for pl in tile_pieces))
    if key not in _PROGRAM_CACHE:
        _PROGRAM_CACHE.clear()
        _PROGRAM_CACHE[key] = _build_program(chains, tile_pieces, NT, NPP)
    nc = _PROGRAM_CACHE[key]

    x_bf = x.astype(ml_dtypes.bfloat16)
    iota = np.broadcast_to(np.arange(P).astype(ml_dtypes.bfloat16),
                           (P, P)).copy()
    ident = np.eye(P, dtype=np.float32)
    biasb = np.broadcast_to(bias, (P, F_OUT)).copy()
    maps = []
    for c in range(N_CORES):
        IDX, DL, VL = per_core[c]
        maps.append({"x": x_bf, "w": W, "iota": iota, "ident": ident,
                     "biasb": biasb, "idx": IDX, "dstloc": DL, "vals": VL})

    trace = os.environ.get("GCN_KERNEL_TRACE", "0") == "1"
    if trace:
        _install_profile_shim()
    from concourse.bass_utils import run_bass_kernel_spmd
    res = run_bass_kernel_spmd(nc, maps, list(range(N_CORES)), trace=trace)
    if trace and res.exec_time_ns is not None:
        print(f"HW exec time: {res.exec_time_ns} ns")

    out = np.empty((N_NODES, F_OUT), dtype=np.float32)
    for c in range(N_CORES):
        r = res.results[c]["out"]
        for s in range(NBLK):
            blk = int(perms[c, s])
            rows = min(P, NPC - blk * P)
            out[c * NPC + blk * P: c * NPC + blk * P + rows, :] = \
                r[s * P: s * P + rows, :]
    return out
